# revision 13
# baseline (speedup 1.0000x reference)
"""GCN (PyG GCNConv + 3-layer MLP + log_softmax) on 8 Trainium2 NeuronCores.

Strategy (graph/data parallel, no collectives):
  * Nodes are bin-packed into (core, group) bins of <=128 nodes, balanced by
    in-degree so every group has a near-equal edge count.
  * Aggregation runs in x-space (A_hat @ (x W) == (A_hat @ x) W), so the
    irregular gather moves 128-float rows instead of 256-float rows.
  * The per-edge gather of x[src] rows uses the Q7 `dma_gather` extended
    instruction (int16 indices), with x split into 4 row-ranges of 25000 so
    indices fit int16.  One dma_gather per (7-group block, range) = 56 ops
    per core; each group's edges are padded to 3 tiles (384 slots) per range
    so every 128-edge tile is group-pure at a compile-time location.
  * Scatter-add per group is a one-hot matmul: S_T[e, n] = (slot(dst_e)==n)
    * norm_e built by one dual-op tensor_scalar; aggT accumulates in PSUM.
  * The MLP runs in transposed [feat, nodes] layout (biases become
    per-partition scalars on the scalar engine); logits are PE-transposed
    back and log_softmax runs along the free dim.
"""

import sys

sys.path.insert(0, "/opt/trn_rl_repo")

import math

import numpy as np

import concourse.bass as bass
import concourse.bacc as bacc
import concourse.mybir as mybir
import concourse.tile as tile
from concourse.masks import make_identity
from concourse.bass_utils import run_bass_kernel_spmd

P = 128
N_NODES = 100000
N_EDGES = 800000
F_IN = 128
F_HID = 256
N_CLS = 40
N_CORES = 8
G_GROUPS = 98        # groups of 128 node slots per core; 98*128 = 12544 >= 12500
BLK = 7              # groups per gather block
N_BLKS = G_GROUPS // BLK  # 14
N_RANGES = 4
RANGE_SZ = 25000     # x row-range per dma_gather table (int16-indexable)
TPR = 3              # tiles per (group, range): quota 384 edges
QUOTA = TPR * P      # 384
TPG = N_RANGES * TPR  # 12 tiles per group
OP_IDXS = BLK * QUOTA          # 2688 idxs per dma_gather op
OP_IDXCOLS = OP_IDXS // 16     # 168 int16 cols per op
N_OPS = N_BLKS * N_RANGES      # 56 gather ops per core

f32 = mybir.dt.float32
i16 = mybir.dt.int16


def build_program(g_groups, n_cores):
    nc = bacc.Bacc(
        "TRN2", target_bir_lowering=False, debug=False, num_devices=n_cores
    )
    G = g_groups

    xr = [
        nc.dram_tensor(f"x{r}", [RANGE_SZ, F_IN], f32, kind="ExternalInput").ap()
        for r in range(N_RANGES)
    ]
    eidx = nc.dram_tensor("eidx", [P, N_OPS * OP_IDXCOLS], i16, kind="ExternalInput").ap()
    erel = nc.dram_tensor("erel", [P, G * TPG], f32, kind="ExternalInput").ap()
    enorm = nc.dram_tensor("enorm", [P, G * TPG], f32, kind="ExternalInput").ap()
    w_in = nc.dram_tensor("w_in", [F_IN, F_HID], f32, kind="ExternalInput").ap()
    w1_in = nc.dram_tensor("w1_in", [F_HID, F_HID // 2], f32, kind="ExternalInput").ap()
    w2_in = nc.dram_tensor("w2_in", [F_HID // 2, F_HID // 4], f32, kind="ExternalInput").ap()
    w3_in = nc.dram_tensor("w3_in", [F_HID // 4, N_CLS], f32, kind="ExternalInput").ap()
    b_in = nc.dram_tensor("b_in", [F_HID, 1], f32, kind="ExternalInput").ap()
    b1_in = nc.dram_tensor("b1_in", [F_HID // 2, 1], f32, kind="ExternalInput").ap()
    b2_in = nc.dram_tensor("b2_in", [F_HID // 4, 1], f32, kind="ExternalInput").ap()
    b3_in = nc.dram_tensor("b3_in", [N_CLS, 1], f32, kind="ExternalInput").ap()
    iota_in = nc.dram_tensor("iota_in", [P, P], f32, kind="ExternalInput").ap()
    out = nc.dram_tensor("out", [G * P, N_CLS], f32, kind="ExternalOutput").ap()

    with tile.TileContext(nc) as tc:
        with (
            tc.tile_pool(name="const", bufs=1) as cpool,
            tc.tile_pool(name="gath", bufs=3) as gpool,
            tc.tile_pool(name="sel", bufs=6) as spool,
            tc.tile_pool(name="act", bufs=3) as mpool,
            tc.tile_pool(name="stat", bufs=4) as vpool,
            tc.tile_pool(name="pagg", bufs=3, space="PSUM") as psum_agg,
            tc.tile_pool(name="pmlp", bufs=4, space="PSUM") as psum_mlp,
        ):
            # ---- constants / metadata, loaded once
            wt = cpool.tile([F_IN, F_HID], f32)
            nc.sync.dma_start(out=wt[:], in_=w_in[:])
            w1a = cpool.tile([P, P], f32, tag="w1a")
            nc.sync.dma_start(out=w1a[:], in_=w1_in[0:P, :])
            w1b = cpool.tile([P, P], f32, tag="w1b")
            nc.sync.dma_start(out=w1b[:], in_=w1_in[P : 2 * P, :])
            w2 = cpool.tile([P, F_HID // 4], f32, tag="w2")
            nc.sync.dma_start(out=w2[:], in_=w2_in[:])
            w3 = cpool.tile([F_HID // 4, N_CLS], f32, tag="w3")
            nc.sync.dma_start(out=w3[:], in_=w3_in[:])
            ba = cpool.tile([P, 1], f32, tag="ba")
            nc.sync.dma_start(out=ba[:], in_=b_in[0:P, :])
            bb = cpool.tile([P, 1], f32, tag="bb")
            nc.sync.dma_start(out=bb[:], in_=b_in[P : 2 * P, :])
            b1t = cpool.tile([P, 1], f32, tag="b1t")
            nc.sync.dma_start(out=b1t[:], in_=b1_in[:])
            b2t = cpool.tile([F_HID // 4, 1], f32, tag="b2t")
            nc.sync.dma_start(out=b2t[:], in_=b2_in[:])
            b3t = cpool.tile([N_CLS, 1], f32, tag="b3t")
            nc.sync.dma_start(out=b3t[:], in_=b3_in[:])
            iota = cpool.tile([P, P], f32, tag="iota")
            nc.sync.dma_start(out=iota[:], in_=iota_in[:])
            ident = cpool.tile([P, P], f32, tag="ident")
            make_identity(nc, ident[:])
            zcol = cpool.tile([P, 1], f32, tag="zcol")
            nc.vector.memset(zcol[:], 0.0)
            eidx_t = cpool.tile([P, N_OPS * OP_IDXCOLS], i16, tag="eidx")
            nc.sync.dma_start(out=eidx_t[:], in_=eidx[:])
            erel_t = cpool.tile([P, G * TPG], f32, tag="erel")
            nc.sync.dma_start(out=erel_t[:], in_=erel[:])
            enorm_t = cpool.tile([P, G * TPG], f32, tag="enorm")
            nc.sync.dma_start(out=enorm_t[:], in_=enorm[:])

            for b in range(N_BLKS):
                gts = []
                for r in range(N_RANGES):
                    op_i = b * N_RANGES + r
                    gt = gpool.tile([P, BLK * TPR, P], f32, tag=f"g{r}")
                    nc.gpsimd.dma_gather(
                        gt[:],
                        xr[r][:],
                        eidx_t[:, op_i * OP_IDXCOLS : (op_i + 1) * OP_IDXCOLS],
                        OP_IDXS,
                        OP_IDXS,
                        F_IN,
                        single_packet=False,
                    )
                    gts.append(gt)
                for gl in range(BLK):
                    g = b * BLK + gl
                    # aggT[f, n] = sum_e msg[e, f] * S_T[e, n], 12 tiles
                    aggp = psum_agg.tile([P, P], f32, tag="aggp")
                    for r in range(N_RANGES):
                        for t in range(TPR):
                            col = g * TPG + r * TPR + t
                            st01 = spool.tile([P, P], f32, tag="st01")
                            nc.vector.tensor_tensor(
                                out=st01[:],
                                in0=erel_t[:, col : col + 1].to_broadcast([P, P]),
                                in1=iota[:],
                                op=mybir.AluOpType.is_equal,
                            )
                            st = spool.tile([P, P], f32, tag="st")
                            nc.vector.tensor_tensor(
                                out=st[:],
                                in0=enorm_t[:, col : col + 1].to_broadcast([P, P]),
                                in1=st01[:],
                                op=mybir.AluOpType.mult,
                            )
                            nc.tensor.matmul(
                                out=aggp[:],
                                lhsT=gts[r][:, gl * TPR + t, :],
                                rhs=st[:],
                                start=(r == 0 and t == 0),
                                stop=(r == N_RANGES - 1 and t == TPR - 1),
                            )
                    aggs = mpool.tile([P, P], f32, tag="aggs")
                    nc.vector.tensor_copy(aggs[:], aggp[:])

                    # layer 1: hT = relu(W^T aggT + b), two 128-row halves
                    hs = []
                    for half in range(2):
                        hp = psum_mlp.tile([P, P], f32, tag="pmlp")
                        nc.tensor.matmul(
                            out=hp[:],
                            lhsT=wt[:, half * P : (half + 1) * P],
                            rhs=aggs[:],
                            start=True,
                            stop=True,
                        )
                        h = mpool.tile([P, P], f32, tag=f"h{half}")
                        nc.scalar.activation(
                            out=h[:],
                            in_=hp[:],
                            func=mybir.ActivationFunctionType.Relu,
                            bias=(ba if half == 0 else bb)[:],
                        )
                        hs.append(h)

                    # layer 2: h1T = relu(W1^T hT + b1), K=256 via 2 matmuls
                    h1p = psum_mlp.tile([P, P], f32, tag="pmlp")
                    nc.tensor.matmul(out=h1p[:], lhsT=w1a[:], rhs=hs[0][:], start=True, stop=False)
                    nc.tensor.matmul(out=h1p[:], lhsT=w1b[:], rhs=hs[1][:], start=False, stop=True)
                    h1 = mpool.tile([P, P], f32, tag="h1")
                    nc.scalar.activation(
                        out=h1[:], in_=h1p[:],
                        func=mybir.ActivationFunctionType.Relu, bias=b1t[:],
                    )

                    # layer 3: h2T = relu(W2^T h1T + b2)  [64, 128]
                    h2p = psum_mlp.tile([F_HID // 4, P], f32, tag="pmlp")
                    nc.tensor.matmul(out=h2p[:], lhsT=w2[:], rhs=h1[:], start=True, stop=True)
                    h2 = mpool.tile([F_HID // 4, P], f32, tag="h2")
                    nc.scalar.activation(
                        out=h2[:], in_=h2p[:],
                        func=mybir.ActivationFunctionType.Relu, bias=b2t[:],
                    )

                    # layer 4: logitsT = W3^T h2T + b3  [40, 128]
                    lp = psum_mlp.tile([N_CLS, P], f32, tag="pmlp")
                    nc.tensor.matmul(out=lp[:], lhsT=w3[:], rhs=h2[:], start=True, stop=True)
                    ls = mpool.tile([N_CLS, P], f32, tag="ls")
                    nc.scalar.activation(
                        out=ls[:], in_=lp[:],
                        func=mybir.ActivationFunctionType.Identity, bias=b3t[:],
                    )

                    # transpose logits to [nodes, cls]
                    tp = psum_mlp.tile([P, N_CLS], f32, tag="pmlp")
                    nc.tensor.transpose(out=tp[:], in_=ls[:], identity=ident[:N_CLS, :N_CLS])

                    # log_softmax along free dim
                    mx = vpool.tile([P, 1], f32, tag="mx")
                    nc.vector.tensor_reduce(
                        out=mx[:], in_=tp[:], axis=mybir.AxisListType.X,
                        op=mybir.AluOpType.max,
                    )
                    nmx = vpool.tile([P, 1], f32, tag="nmx")
                    nc.vector.tensor_tensor(
                        out=nmx[:], in0=zcol[:], in1=mx[:],
                        op=mybir.AluOpType.subtract,
                    )
                    et = vpool.tile([P, N_CLS], f32, tag="et")
                    nc.scalar.activation(
                        out=et[:], in_=tp[:],
                        func=mybir.ActivationFunctionType.Exp, bias=nmx[:],
                    )
                    sm = vpool.tile([P, 1], f32, tag="sm")
                    nc.vector.tensor_reduce(
                        out=sm[:], in_=et[:], axis=mybir.AxisListType.X,
                        op=mybir.AluOpType.add,
                    )
                    lse = vpool.tile([P, 1], f32, tag="lse")
                    nc.scalar.activation(
                        out=lse[:], in_=sm[:], func=mybir.ActivationFunctionType.Ln,
                    )
                    nc2 = vpool.tile([P, 1], f32, tag="nc2")
                    nc.vector.tensor_tensor(
                        out=nc2[:], in0=nmx[:], in1=lse[:],
                        op=mybir.AluOpType.subtract,
                    )
                    ot = vpool.tile([P, N_CLS], f32, tag="ot")
                    nc.vector.tensor_tensor(
                        out=ot[:], in0=nc2[:].to_broadcast([P, N_CLS]), in1=tp[:],
                        op=mybir.AluOpType.add,
                    )
                    nc.sync.dma_start(out=out[g * P : (g + 1) * P, :], in_=ot[:])

    nc.compile()
    return nc


_PROGRAM_CACHE: dict = {}
RUN_KWARGS: dict = {}  # e.g. {"trace": True} — set by test harness before kernel()
LAST_RESULTS = None


def _get_program():
    key = (G_GROUPS, N_CORES)
    if key not in _PROGRAM_CACHE:
        _PROGRAM_CACHE[key] = build_program(G_GROUPS, N_CORES)
    return _PROGRAM_CACHE[key]


def prep_host(x, edge_index, n_cores=N_CORES, g_groups=G_GROUPS):
    """Bin-pack nodes, build per-core gather indices + edge-tile metadata."""
    n = x.shape[0]
    src = np.asarray(edge_index[0], dtype=np.int64)
    dst = np.asarray(edge_index[1], dtype=np.int64)

    deg = (np.bincount(dst, minlength=n) + 1).astype(np.float32)
    dinv = (1.0 / np.sqrt(deg)).astype(np.float32)

    loop = np.arange(n, dtype=np.int64)
    src_all = np.concatenate([src, loop])
    dst_all = np.concatenate([dst, loop])
    norm_all = dinv[src_all] * dinv[dst_all]

    nbins = n_cores * g_groups
    # serpentine assignment of degree-sorted nodes -> near-equal edge load/bin
    order = np.argsort(-deg, kind="stable")
    nodebin = np.empty(n, dtype=np.int64)
    fwd = np.arange(nbins)
    rounds = math.ceil(n / nbins)
    for r in range(rounds):
        chunk = order[r * nbins : (r + 1) * nbins]
        lanes = fwd[: len(chunk)] if r % 2 == 0 else (nbins - 1 - fwd[: len(chunk)])
        nodebin[chunk] = lanes

    perm = np.argsort(nodebin, kind="stable")
    counts = np.bincount(nodebin, minlength=nbins)
    assert counts.max() <= P
    starts = np.concatenate([[0], np.cumsum(counts)[:-1]])
    slot = np.empty(n, dtype=np.int64)
    slot[perm] = np.arange(n) - np.repeat(starts, counts)

    # per-edge coordinates
    ebin = nodebin[dst_all]                     # 0..nbins-1
    erange = src_all // RANGE_SZ                # 0..3
    cell = ebin * N_RANGES + erange             # (bin, range) cell
    ncells = nbins * N_RANGES
    eorder = np.argsort(cell, kind="stable")
    ccounts = np.bincount(cell, minlength=ncells)
    qmax = int(ccounts.max())
    assert qmax <= QUOTA, f"cell overflow: {qmax} > {QUOTA}"
    cstarts = np.concatenate([[0], np.cumsum(ccounts)[:-1]])
    q = np.empty(len(cell), dtype=np.int64)
    q[eorder] = np.arange(len(cell)) - np.repeat(cstarts, ccounts)

    core = ebin // g_groups
    grp = ebin % g_groups
    t = q // P
    pp = q % P
    col = grp * TPG + erange * TPR + t

    erel = np.zeros((n_cores, P, g_groups * TPG), dtype=np.float32)
    enorm = np.zeros((n_cores, P, g_groups * TPG), dtype=np.float32)
    erel[core, pp, col] = slot[dst_all].astype(np.float32)
    enorm[core, pp, col] = norm_all

    # gather idx (int16 into the per-range table), in flat (op, j) layout:
    # op = (blk, range); j = ((grp % BLK) * TPR + t) * 128 + pp
    flat = np.zeros((n_cores, N_OPS, OP_IDXS), dtype=np.int16)
    op_i = (grp // BLK) * N_RANGES + erange
    j = ((grp % BLK) * TPR + t) * P + pp
    flat[core, op_i, j] = (src_all % RANGE_SZ).astype(np.int16)

    # wrap: idx j -> partition j%16, col j//16; replicate x8 across partitions
    w = flat.reshape(n_cores, N_OPS, OP_IDXCOLS, 16)     # j = c*16 + p
    w = np.transpose(w, (0, 3, 1, 2))                    # [cores, 16, N_OPS, cols]
    w = w.reshape(n_cores, 16, N_OPS * OP_IDXCOLS)
    eidx = np.tile(w, (1, 8, 1))                         # [cores, 128, N_OPS*cols]

    return dict(
        eidx=np.ascontiguousarray(eidx),
        erel=erel,
        enorm=enorm,
        nodebin=nodebin,
        slot=slot,
    )


def kernel(x, edge_index, W, b, W1, b1, W2, b2, W3, b3):
    x = np.ascontiguousarray(np.asarray(x, dtype=np.float32))
    n = x.shape[0]
    meta = prep_host(x, edge_index)

    nc = _get_program()

    iota = np.tile(np.arange(P, dtype=np.float32), (P, 1))
    common = {
        "w_in": np.asarray(W, dtype=np.float32),
        "w1_in": np.asarray(W1, dtype=np.float32),
        "w2_in": np.asarray(W2, dtype=np.float32),
        "w3_in": np.asarray(W3, dtype=np.float32),
        "b_in": np.asarray(b, dtype=np.float32).reshape(-1, 1),
        "b1_in": np.asarray(b1, dtype=np.float32).reshape(-1, 1),
        "b2_in": np.asarray(b2, dtype=np.float32).reshape(-1, 1),
        "b3_in": np.asarray(b3, dtype=np.float32).reshape(-1, 1),
        "iota_in": iota,
    }
    for r in range(N_RANGES):
        common[f"x{r}"] = np.ascontiguousarray(x[r * RANGE_SZ : (r + 1) * RANGE_SZ])
    in_maps = []
    for c in range(N_CORES):
        m = dict(common)
        m["eidx"] = meta["eidx"][c]
        m["erel"] = meta["erel"][c]
        m["enorm"] = meta["enorm"][c]
        in_maps.append(m)

    global LAST_RESULTS
    LAST_RESULTS = run_bass_kernel_spmd(
        nc, in_maps, list(range(N_CORES)), **RUN_KWARGS
    )
    res = LAST_RESULTS.results

    nodebin = meta["nodebin"]
    slot = meta["slot"]
    core = nodebin // G_GROUPS
    row = (nodebin % G_GROUPS) * P + slot
    out_full = np.empty((n, N_CLS), dtype=np.float32)
    for c in range(N_CORES):
        mask = core == c
        out_full[mask] = res[c]["out"][row[mask]]
    return out_full


# revision 14
# speedup vs baseline: 1.0081x; 1.0081x over previous
"""GCN (PyG GCNConv + 3-layer MLP + log_softmax) on 8 Trainium2 NeuronCores.

Strategy (graph/data parallel, no collectives):
  * Nodes are bin-packed into (core, group) bins of <=128 nodes, balanced by
    in-degree so every group has a near-equal edge count.
  * Aggregation runs in x-space (A_hat @ (x W) == (A_hat @ x) W), so the
    irregular gather moves 128-float rows instead of 256-float rows.
  * The per-edge gather of x[src] rows uses the Q7 `dma_gather` extended
    instruction (int16 indices), with x split into 4 row-ranges of 25000 so
    indices fit int16.  One dma_gather per (7-group block, range) = 56 ops
    per core; each group's edges are padded to 3 tiles (384 slots) per range
    so every 128-edge tile is group-pure at a compile-time location.
  * Scatter-add per group is a one-hot matmul: S_T[e, n] = (slot(dst_e)==n)
    * norm_e built by one dual-op tensor_scalar; aggT accumulates in PSUM.
  * The MLP runs in transposed [feat, nodes] layout (biases become
    per-partition scalars on the scalar engine); logits are PE-transposed
    back and log_softmax runs along the free dim.
"""

import sys

sys.path.insert(0, "/opt/trn_rl_repo")

import math

import numpy as np

import concourse.bass as bass
import concourse.bacc as bacc
import concourse.mybir as mybir
import concourse.tile as tile
from concourse.masks import make_identity
from concourse.bass_utils import run_bass_kernel_spmd

P = 128
N_NODES = 100000
N_EDGES = 800000
F_IN = 128
F_HID = 256
N_CLS = 40
N_CORES = 8
G_GROUPS = 98        # groups of 128 node slots per core; 98*128 = 12544 >= 12500
BLK = 7              # groups per gather block
N_BLKS = G_GROUPS // BLK  # 14
N_RANGES = 4
RANGE_SZ = 25000     # x row-range per dma_gather table (int16-indexable)
TPR = 3              # tiles per (group, range): quota 384 edges
QUOTA = TPR * P      # 384
TPG = N_RANGES * TPR  # 12 tiles per group
OP_IDXS = BLK * QUOTA          # 2688 idxs per dma_gather op
OP_IDXCOLS = OP_IDXS // 16     # 168 int16 cols per op
N_OPS = N_BLKS * N_RANGES      # 56 gather ops per core

f32 = mybir.dt.float32
i16 = mybir.dt.int16


def build_program(g_groups, n_cores):
    nc = bacc.Bacc(
        "TRN2", target_bir_lowering=False, debug=False, num_devices=n_cores
    )
    G = g_groups

    xr = [
        nc.dram_tensor(f"x{r}", [RANGE_SZ, F_IN], f32, kind="ExternalInput").ap()
        for r in range(N_RANGES)
    ]
    eidx = nc.dram_tensor("eidx", [P, N_OPS * OP_IDXCOLS], i16, kind="ExternalInput").ap()
    erel = nc.dram_tensor("erel", [P, G * TPG], f32, kind="ExternalInput").ap()
    enorm = nc.dram_tensor("enorm", [P, G * TPG], f32, kind="ExternalInput").ap()
    w_in = nc.dram_tensor("w_in", [F_IN, F_HID], f32, kind="ExternalInput").ap()
    w1_in = nc.dram_tensor("w1_in", [F_HID, F_HID // 2], f32, kind="ExternalInput").ap()
    w2_in = nc.dram_tensor("w2_in", [F_HID // 2, F_HID // 4], f32, kind="ExternalInput").ap()
    w3_in = nc.dram_tensor("w3_in", [F_HID // 4, N_CLS], f32, kind="ExternalInput").ap()
    b_in = nc.dram_tensor("b_in", [F_HID, 1], f32, kind="ExternalInput").ap()
    b1_in = nc.dram_tensor("b1_in", [F_HID // 2, 1], f32, kind="ExternalInput").ap()
    b2_in = nc.dram_tensor("b2_in", [F_HID // 4, 1], f32, kind="ExternalInput").ap()
    b3_in = nc.dram_tensor("b3_in", [N_CLS, 1], f32, kind="ExternalInput").ap()
    iota_in = nc.dram_tensor("iota_in", [P, P], f32, kind="ExternalInput").ap()
    out = nc.dram_tensor("out", [G * P, N_CLS], f32, kind="ExternalOutput").ap()

    with tile.TileContext(nc) as tc:
        with (
            tc.tile_pool(name="const", bufs=1) as cpool,
            tc.tile_pool(name="gath", bufs=2) as gpool,
            tc.tile_pool(name="sel", bufs=4) as spool,
            tc.tile_pool(name="act", bufs=3) as mpool,
            tc.tile_pool(name="stat", bufs=4) as vpool,
            tc.tile_pool(name="pagg", bufs=2, space="PSUM") as psum_agg,
            tc.tile_pool(name="pmlp", bufs=4, space="PSUM") as psum_mlp,
        ):
            # ---- constants / metadata, loaded once
            wt = cpool.tile([F_IN, F_HID], f32)
            nc.sync.dma_start(out=wt[:], in_=w_in[:])
            w1a = cpool.tile([P, P], f32, tag="w1a")
            nc.sync.dma_start(out=w1a[:], in_=w1_in[0:P, :])
            w1b = cpool.tile([P, P], f32, tag="w1b")
            nc.sync.dma_start(out=w1b[:], in_=w1_in[P : 2 * P, :])
            w2 = cpool.tile([P, F_HID // 4], f32, tag="w2")
            nc.sync.dma_start(out=w2[:], in_=w2_in[:])
            w3 = cpool.tile([F_HID // 4, N_CLS], f32, tag="w3")
            nc.sync.dma_start(out=w3[:], in_=w3_in[:])
            ba = cpool.tile([P, 1], f32, tag="ba")
            nc.sync.dma_start(out=ba[:], in_=b_in[0:P, :])
            bb = cpool.tile([P, 1], f32, tag="bb")
            nc.sync.dma_start(out=bb[:], in_=b_in[P : 2 * P, :])
            b1t = cpool.tile([P, 1], f32, tag="b1t")
            nc.sync.dma_start(out=b1t[:], in_=b1_in[:])
            b2t = cpool.tile([F_HID // 4, 1], f32, tag="b2t")
            nc.sync.dma_start(out=b2t[:], in_=b2_in[:])
            b3t = cpool.tile([N_CLS, 1], f32, tag="b3t")
            nc.sync.dma_start(out=b3t[:], in_=b3_in[:])
            iota = cpool.tile([P, P], f32, tag="iota")
            nc.sync.dma_start(out=iota[:], in_=iota_in[:])
            ident = cpool.tile([P, P], f32, tag="ident")
            make_identity(nc, ident[:])
            zcol = cpool.tile([P, 1], f32, tag="zcol")
            nc.vector.memset(zcol[:], 0.0)
            eidx_t = cpool.tile([P, N_OPS * OP_IDXCOLS], i16, tag="eidx")
            nc.sync.dma_start(out=eidx_t[:], in_=eidx[:])
            erel_t = cpool.tile([P, G * TPG], f32, tag="erel")
            nc.sync.dma_start(out=erel_t[:], in_=erel[:])
            enorm_t = cpool.tile([P, G * TPG], f32, tag="enorm")
            nc.sync.dma_start(out=enorm_t[:], in_=enorm[:])

            for b in range(N_BLKS):
                gts = []
                for r in range(N_RANGES):
                    op_i = b * N_RANGES + r
                    gt = gpool.tile([P, BLK * TPR, P], f32, tag=f"g{r}")
                    nc.gpsimd.dma_gather(
                        gt[:],
                        xr[r][:],
                        eidx_t[:, op_i * OP_IDXCOLS : (op_i + 1) * OP_IDXCOLS],
                        OP_IDXS,
                        OP_IDXS,
                        F_IN,
                        single_packet=False,
                    )
                    gts.append(gt)
                for gl in range(BLK):
                    g = b * BLK + gl
                    # aggT[f, n] = sum_e msg[e, f] * S_T[e, n], 12 tiles
                    aggp = psum_agg.tile([P, P], f32, tag="aggp")
                    for r in range(N_RANGES):
                        for t in range(TPR):
                            col = g * TPG + r * TPR + t
                            st01 = spool.tile([P, P], f32, tag="st01")
                            nc.vector.tensor_tensor(
                                out=st01[:],
                                in0=erel_t[:, col : col + 1].to_broadcast([P, P]),
                                in1=iota[:],
                                op=mybir.AluOpType.is_equal,
                            )
                            st = spool.tile([P, P], f32, tag="st")
                            nc.vector.tensor_tensor(
                                out=st[:],
                                in0=enorm_t[:, col : col + 1].to_broadcast([P, P]),
                                in1=st01[:],
                                op=mybir.AluOpType.mult,
                            )
                            nc.tensor.matmul(
                                out=aggp[:],
                                lhsT=gts[r][:, gl * TPR + t, :],
                                rhs=st[:],
                                start=(r == 0 and t == 0),
                                stop=(r == N_RANGES - 1 and t == TPR - 1),
                            )
                    aggs = mpool.tile([P, P], f32, tag="aggs")
                    nc.vector.tensor_copy(aggs[:], aggp[:])

                    # layer 1: hT = relu(W^T aggT + b), two 128-row halves
                    hs = []
                    for half in range(2):
                        hp = psum_mlp.tile([P, P], f32, tag="pmlp")
                        nc.tensor.matmul(
                            out=hp[:],
                            lhsT=wt[:, half * P : (half + 1) * P],
                            rhs=aggs[:],
                            start=True,
                            stop=True,
                        )
                        h = mpool.tile([P, P], f32, tag=f"h{half}")
                        nc.scalar.activation(
                            out=h[:],
                            in_=hp[:],
                            func=mybir.ActivationFunctionType.Relu,
                            bias=(ba if half == 0 else bb)[:],
                        )
                        hs.append(h)

                    # layer 2: h1T = relu(W1^T hT + b1), K=256 via 2 matmuls
                    h1p = psum_mlp.tile([P, P], f32, tag="pmlp")
                    nc.tensor.matmul(out=h1p[:], lhsT=w1a[:], rhs=hs[0][:], start=True, stop=False)
                    nc.tensor.matmul(out=h1p[:], lhsT=w1b[:], rhs=hs[1][:], start=False, stop=True)
                    h1 = mpool.tile([P, P], f32, tag="h1")
                    nc.scalar.activation(
                        out=h1[:], in_=h1p[:],
                        func=mybir.ActivationFunctionType.Relu, bias=b1t[:],
                    )

                    # layer 3: h2T = relu(W2^T h1T + b2)  [64, 128]
                    h2p = psum_mlp.tile([F_HID // 4, P], f32, tag="pmlp")
                    nc.tensor.matmul(out=h2p[:], lhsT=w2[:], rhs=h1[:], start=True, stop=True)
                    h2 = mpool.tile([F_HID // 4, P], f32, tag="h2")
                    nc.scalar.activation(
                        out=h2[:], in_=h2p[:],
                        func=mybir.ActivationFunctionType.Relu, bias=b2t[:],
                    )

                    # layer 4: logitsT = W3^T h2T + b3  [40, 128]
                    lp = psum_mlp.tile([N_CLS, P], f32, tag="pmlp")
                    nc.tensor.matmul(out=lp[:], lhsT=w3[:], rhs=h2[:], start=True, stop=True)
                    ls = mpool.tile([N_CLS, P], f32, tag="ls")
                    nc.scalar.activation(
                        out=ls[:], in_=lp[:],
                        func=mybir.ActivationFunctionType.Identity, bias=b3t[:],
                    )

                    # transpose logits to [nodes, cls]
                    tp = psum_mlp.tile([P, N_CLS], f32, tag="pmlp")
                    nc.tensor.transpose(out=tp[:], in_=ls[:], identity=ident[:N_CLS, :N_CLS])

                    # log_softmax along free dim
                    mx = vpool.tile([P, 1], f32, tag="mx")
                    nc.vector.tensor_reduce(
                        out=mx[:], in_=tp[:], axis=mybir.AxisListType.X,
                        op=mybir.AluOpType.max,
                    )
                    nmx = vpool.tile([P, 1], f32, tag="nmx")
                    nc.vector.tensor_tensor(
                        out=nmx[:], in0=zcol[:], in1=mx[:],
                        op=mybir.AluOpType.subtract,
                    )
                    et = vpool.tile([P, N_CLS], f32, tag="et")
                    nc.scalar.activation(
                        out=et[:], in_=tp[:],
                        func=mybir.ActivationFunctionType.Exp, bias=nmx[:],
                    )
                    sm = vpool.tile([P, 1], f32, tag="sm")
                    nc.vector.tensor_reduce(
                        out=sm[:], in_=et[:], axis=mybir.AxisListType.X,
                        op=mybir.AluOpType.add,
                    )
                    lse = vpool.tile([P, 1], f32, tag="lse")
                    nc.scalar.activation(
                        out=lse[:], in_=sm[:], func=mybir.ActivationFunctionType.Ln,
                    )
                    nc2 = vpool.tile([P, 1], f32, tag="nc2")
                    nc.vector.tensor_tensor(
                        out=nc2[:], in0=nmx[:], in1=lse[:],
                        op=mybir.AluOpType.subtract,
                    )
                    ot = vpool.tile([P, N_CLS], f32, tag="ot")
                    nc.vector.tensor_tensor(
                        out=ot[:], in0=nc2[:].to_broadcast([P, N_CLS]), in1=tp[:],
                        op=mybir.AluOpType.add,
                    )
                    nc.sync.dma_start(out=out[g * P : (g + 1) * P, :], in_=ot[:])

    nc.compile()
    return nc


_PROGRAM_CACHE: dict = {}
RUN_KWARGS: dict = {}  # e.g. {"trace": True} — set by test harness before kernel()
LAST_RESULTS = None


def _get_program():
    key = (G_GROUPS, N_CORES)
    if key not in _PROGRAM_CACHE:
        _PROGRAM_CACHE[key] = build_program(G_GROUPS, N_CORES)
    return _PROGRAM_CACHE[key]


def prep_host(x, edge_index, n_cores=N_CORES, g_groups=G_GROUPS):
    """Bin-pack nodes, build per-core gather indices + edge-tile metadata."""
    n = x.shape[0]
    src = np.asarray(edge_index[0], dtype=np.int64)
    dst = np.asarray(edge_index[1], dtype=np.int64)

    deg = (np.bincount(dst, minlength=n) + 1).astype(np.float32)
    dinv = (1.0 / np.sqrt(deg)).astype(np.float32)

    loop = np.arange(n, dtype=np.int64)
    src_all = np.concatenate([src, loop])
    dst_all = np.concatenate([dst, loop])
    norm_all = dinv[src_all] * dinv[dst_all]

    nbins = n_cores * g_groups
    # serpentine assignment of degree-sorted nodes -> near-equal edge load/bin
    order = np.argsort(-deg, kind="stable")
    nodebin = np.empty(n, dtype=np.int64)
    fwd = np.arange(nbins)
    rounds = math.ceil(n / nbins)
    for r in range(rounds):
        chunk = order[r * nbins : (r + 1) * nbins]
        lanes = fwd[: len(chunk)] if r % 2 == 0 else (nbins - 1 - fwd[: len(chunk)])
        nodebin[chunk] = lanes

    perm = np.argsort(nodebin, kind="stable")
    counts = np.bincount(nodebin, minlength=nbins)
    assert counts.max() <= P
    starts = np.concatenate([[0], np.cumsum(counts)[:-1]])
    slot = np.empty(n, dtype=np.int64)
    slot[perm] = np.arange(n) - np.repeat(starts, counts)

    # per-edge coordinates
    ebin = nodebin[dst_all]                     # 0..nbins-1
    erange = src_all // RANGE_SZ                # 0..3
    cell = ebin * N_RANGES + erange             # (bin, range) cell
    ncells = nbins * N_RANGES
    eorder = np.argsort(cell, kind="stable")
    ccounts = np.bincount(cell, minlength=ncells)
    qmax = int(ccounts.max())
    assert qmax <= QUOTA, f"cell overflow: {qmax} > {QUOTA}"
    cstarts = np.concatenate([[0], np.cumsum(ccounts)[:-1]])
    q = np.empty(len(cell), dtype=np.int64)
    q[eorder] = np.arange(len(cell)) - np.repeat(cstarts, ccounts)

    core = ebin // g_groups
    grp = ebin % g_groups
    t = q // P
    pp = q % P
    col = grp * TPG + erange * TPR + t

    erel = np.zeros((n_cores, P, g_groups * TPG), dtype=np.float32)
    enorm = np.zeros((n_cores, P, g_groups * TPG), dtype=np.float32)
    erel[core, pp, col] = slot[dst_all].astype(np.float32)
    enorm[core, pp, col] = norm_all

    # gather idx (int16 into the per-range table), in flat (op, j) layout:
    # op = (blk, range); j = ((grp % BLK) * TPR + t) * 128 + pp
    flat = np.zeros((n_cores, N_OPS, OP_IDXS), dtype=np.int16)
    op_i = (grp // BLK) * N_RANGES + erange
    j = ((grp % BLK) * TPR + t) * P + pp
    flat[core, op_i, j] = (src_all % RANGE_SZ).astype(np.int16)

    # wrap: idx j -> partition j%16, col j//16; replicate x8 across partitions
    w = flat.reshape(n_cores, N_OPS, OP_IDXCOLS, 16)     # j = c*16 + p
    w = np.transpose(w, (0, 3, 1, 2))                    # [cores, 16, N_OPS, cols]
    w = w.reshape(n_cores, 16, N_OPS * OP_IDXCOLS)
    eidx = np.tile(w, (1, 8, 1))                         # [cores, 128, N_OPS*cols]

    return dict(
        eidx=np.ascontiguousarray(eidx),
        erel=erel,
        enorm=enorm,
        nodebin=nodebin,
        slot=slot,
    )


def kernel(x, edge_index, W, b, W1, b1, W2, b2, W3, b3):
    x = np.ascontiguousarray(np.asarray(x, dtype=np.float32))
    n = x.shape[0]
    meta = prep_host(x, edge_index)

    nc = _get_program()

    iota = np.tile(np.arange(P, dtype=np.float32), (P, 1))
    common = {
        "w_in": np.asarray(W, dtype=np.float32),
        "w1_in": np.asarray(W1, dtype=np.float32),
        "w2_in": np.asarray(W2, dtype=np.float32),
        "w3_in": np.asarray(W3, dtype=np.float32),
        "b_in": np.asarray(b, dtype=np.float32).reshape(-1, 1),
        "b1_in": np.asarray(b1, dtype=np.float32).reshape(-1, 1),
        "b2_in": np.asarray(b2, dtype=np.float32).reshape(-1, 1),
        "b3_in": np.asarray(b3, dtype=np.float32).reshape(-1, 1),
        "iota_in": iota,
    }
    for r in range(N_RANGES):
        common[f"x{r}"] = np.ascontiguousarray(x[r * RANGE_SZ : (r + 1) * RANGE_SZ])
    in_maps = []
    for c in range(N_CORES):
        m = dict(common)
        m["eidx"] = meta["eidx"][c]
        m["erel"] = meta["erel"][c]
        m["enorm"] = meta["enorm"][c]
        in_maps.append(m)

    global LAST_RESULTS
    LAST_RESULTS = run_bass_kernel_spmd(
        nc, in_maps, list(range(N_CORES)), **RUN_KWARGS
    )
    res = LAST_RESULTS.results

    nodebin = meta["nodebin"]
    slot = meta["slot"]
    core = nodebin // G_GROUPS
    row = (nodebin % G_GROUPS) * P + slot
    out_full = np.empty((n, N_CLS), dtype=np.float32)
    for c in range(N_CORES):
        mask = core == c
        out_full[mask] = res[c]["out"][row[mask]]
    return out_full
